# revision 1
# baseline (speedup 1.0000x reference)
"""Trainium2 Bass kernel for nn_CTC: Linear projection + log_softmax + CTC loss.

Strategy (8 NeuronCores, data-parallel over batch B=16, 2 rows/core):
- Main projection (hs @ W) in bf16 on TensorE, tiled [128t x 512v] chunks;
  ScalarE computes exp(logit + lnC) in-place on PSUM with a fused free-dim
  accumulate to get per-frame sum-exp tables (log_softmax normalizers).
  Log and masked sums happen on the host in fp64 (tiny data).
- Emission logits for the 2L+1 extended CTC states come from a second small
  matmul against host-gathered W columns (Wg = W[:, ext]).
- The CTC alpha recursion runs in probability domain on VectorE with a
  chunked state layout: state s -> partition c=s//8 (per-b 32-partition
  group), col f=s%8. Cross-chunk halo moves via stream_shuffle (rotate-by-1
  within the 32-group). Numerical range is handled by per-chunk scales:
  every RESC steps each chunk is divided by its own sum (d=1 for dead
  chunks); a per-boundary ratio rho = sigma_{c-1}/sigma_c (clamped) scales
  the halo each step. The per-chunk log-scales are reconstructed on the host
  from the stored d table; emissions fold exp(+lnC) and skip log_softmax
  normalization entirely (logZ is accounted on the host).
- For t >= hlens[b], emissions switch to a synthetic blank-pass pattern
  (blank prob 1, labels 0), which exactly preserves the final
  logaddexp(alpha[2l], alpha[2l-1]) - this replaces the reference's
  per-step framewise masking. Emissions for states beyond 2*ys_lens[b]+1
  are zeroed (they can never reach the answer states).

The program is uniform SPMD; all input-dependent values (hlens masks, label
gathers, skip masks) enter through per-core data tensors built on the host
from the actual inputs at call time. The bias vector b is all-zeros by the
problem's input spec and is not applied.
"""

import numpy as np
import ml_dtypes
from dataclasses import dataclass

import concourse.bass as bass
import concourse.bacc as bacc
import concourse.tile as tile
from concourse import mybir
from concourse.bass_utils import run_bass_kernel_spmd

F32 = mybir.dt.float32
BF16 = mybir.dt.bfloat16
ALU = mybir.AluOpType
AXX = mybir.AxisListType.X
EXP = mybir.ActivationFunctionType.Exp
CPY = mybir.ActivationFunctionType.Copy

NCORES = 8
BPC = 2          # batch rows per core
TBLK = 128


@dataclass
class Cfg:
    T: int = 1000
    TP: int = 1024
    D: int = 512
    V: int = 5000
    L: int = 100
    RESC: int = 16
    LNC: float = -0.9
    CLAMP: float = 1e25
    F: int = 8

    @property
    def NMT(self):
        return self.TP // TBLK

    @property
    def KT(self):
        return self.D // TBLK

    @property
    def S(self):
        return 2 * self.L + 1

    @property
    def NCH(self):
        return (self.S + self.F - 1) // self.F

    @property
    def SP(self):
        return self.NCH * self.F

    @property
    def VCH(self):
        out = []
        v = self.V
        while v > 0:
            out.append(min(512, v))
            v -= out[-1]
        return out

    @property
    def NEV(self):
        return (self.T - 1) // self.RESC

    # packed table offsets (fp32 cols in the tabs tensor)
    @property
    def o_patt(self):
        return 0

    @property
    def o_pk(self):
        return BPC * self.SP

    @property
    def o_skz(self):
        return 2 * BPC * self.SP

    @property
    def o_mh(self):
        return 3 * BPC * self.SP

    @property
    def o_imh(self):
        return self.o_mh + BPC * self.NMT

    @property
    def o_ident(self):
        return self.o_imh + BPC * self.NMT

    @property
    def o_initm(self):
        return self.o_ident + TBLK

    @property
    def o_lnc(self):
        return self.o_initm + self.F

    @property
    def TW(self):
        return self.o_lnc + 1


FULL = Cfg()
ROT1 = [(i - 1) % 32 for i in range(32)]


def build_program(cfg: Cfg, repeats: bool, stage: int = 4, dp_steps: int | None = None) -> bass.Bass:
    c = cfg
    F = c.F
    assert 32 * F == 256 and c.SP <= 256 and c.NCH <= 32
    XZW = 2 * (1 + F) if repeats else 1 + F
    NV = len(c.VCH)
    nc = bacc.Bacc("TRN2", debug=False)

    d_hsT = nc.dram_tensor("hsT", [BPC, c.KT, TBLK, c.TP], BF16, kind="ExternalInput")
    d_W = nc.dram_tensor("Wt", [c.KT, TBLK, c.V], BF16, kind="ExternalInput")
    d_Wg = nc.dram_tensor("Wg", [BPC, c.KT, TBLK, c.SP], BF16, kind="ExternalInput")
    d_tabs = nc.dram_tensor("tabs", [TBLK, c.TW], F32, kind="ExternalInput")
    d_sums = nc.dram_tensor("sums_out", [TBLK, BPC * c.NMT], F32, kind="ExternalOutput")
    d_alpha = nc.dram_tensor("alpha_out", [64, XZW], F32, kind="ExternalOutput")
    d_ctab = nc.dram_tensor("ctab_out", [64, c.NEV], F32, kind="ExternalOutput")

    with tile.TileContext(nc) as tc:
        with (
            tc.tile_pool(name="persist", bufs=1) as pp,
            tc.tile_pool(name="etile", bufs=3) as pe,
            tc.tile_pool(name="csum", bufs=2) as pc,
            tc.tile_pool(name="stgp", bufs=2) as pstg,
            tc.tile_pool(name="mmps", bufs=2, space="PSUM") as pmm,
            tc.tile_pool(name="gps", bufs=2, space="PSUM") as pgp,
            tc.tile_pool(name="tps", bufs=2, space="PSUM") as ptp,
        ):
            # ---- persistent SBUF ----
            sW = pp.tile([TBLK, c.KT * c.V], BF16, tag="sW", name="sW")
            shsT = pp.tile([TBLK, BPC * c.KT * c.TP], BF16, tag="shsT", name="shsT")
            sWg = pp.tile([TBLK, BPC * c.KT * c.SP], BF16, tag="sWg", name="sWg")
            tabs = pp.tile([TBLK, c.TW], F32, tag="tabs", name="tabs")
            e_mt = [pp.tile([64, F * TBLK], F32, tag=f"e_mt{m}", name=f"e_mt{m}")
                    for m in range(c.NMT)]
            ez_mt = ([pp.tile([64, F * TBLK], F32, tag=f"ez_mt{m}", name=f"ez_mt{m}")
                      for m in range(c.NMT)] if repeats else None)
            stab = pp.tile([TBLK, BPC * c.NMT], F32, tag="stab", name="stab")
            xz = pp.tile([64, XZW], F32, tag="xz", name="xz")
            vt = pp.tile([64, F], F32, tag="vt", name="vt")
            rho = pp.tile([64, 1], F32, tag="rho", name="rho")
            tailt = pp.tile([64, 2 if repeats else 1], F32, tag="tailt", name="tailt")
            tot = pp.tile([64, 1], F32, tag="tot", name="tot")
            recip = pp.tile([64, 1], F32, tag="recip", name="recip")
            dsh = pp.tile([64, 1], F32, tag="dsh", name="dsh")
            ctab = pp.tile([64, c.NEV], F32, tag="ctab", name="ctab")

            sident = tabs[:, c.o_ident:c.o_ident + TBLK]
            sinitm = tabs[0:64, c.o_initm:c.o_initm + F]
            slnc = tabs[:, c.o_lnc:c.o_lnc + 1]

            # ---- load inputs ----
            nc.sync.dma_start(tabs[:], d_tabs.ap()[:])
            for k in range(c.KT):
                nc.sync.dma_start(sW[:, k * c.V:(k + 1) * c.V], d_W.ap()[k])
            for b in range(BPC):
                for k in range(c.KT):
                    off = (b * c.KT + k)
                    nc.sync.dma_start(shsT[:, off * c.TP:(off + 1) * c.TP],
                                      d_hsT.ap()[b, k])
                    nc.sync.dma_start(sWg[:, off * c.SP:(off + 1) * c.SP],
                                      d_Wg.ap()[b, k])
            nc.vector.memset(xz[:], 0.0)
            nc.vector.memset(rho[:], 1.0)

            def hs_s(b, k, mt):
                off = (b * c.KT + k) * c.TP + mt * TBLK
                return shsT[:, off:off + TBLK]

            # ---- emission prep per (mt, b) ----
            for mt in range(c.NMT if stage >= 2 else 0):
                for b in range(BPC):
                    psg = pgp.tile([TBLK, c.SP], F32, tag="psg", name="psg")
                    for k in range(c.KT):
                        off = (b * c.KT + k) * c.SP
                        nc.tensor.matmul(psg[:], hs_s(b, k, mt),
                                         sWg[:, off:off + c.SP],
                                         start=(k == 0), stop=(k == c.KT - 1))
                    et = pe.tile([TBLK, c.SP], F32, tag="et", name="et")
                    nc.scalar.activation(et[:], psg[:], EXP, bias=slnc)
                    idx = b * c.NMT + mt
                    pkb = tabs[:, c.o_pk + b * c.SP:c.o_pk + (b + 1) * c.SP]
                    pattb = tabs[:, c.o_patt + b * c.SP:c.o_patt + (b + 1) * c.SP]
                    nc.vector.scalar_tensor_tensor(
                        et[:], et[:], tabs[:, c.o_mh + idx:c.o_mh + idx + 1],
                        pkb, op0=ALU.mult, op1=ALU.mult)
                    nc.vector.scalar_tensor_tensor(
                        et[:], pattb, tabs[:, c.o_imh + idx:c.o_imh + idx + 1],
                        et[:], op0=ALU.mult, op1=ALU.add)
                    tiles = [(et, e_mt)]
                    if repeats:
                        ezt = pe.tile([TBLK, c.SP], F32, tag="ezt", name="ezt")
                        skzb = tabs[:, c.o_skz + b * c.SP:c.o_skz + (b + 1) * c.SP]
                        nc.vector.tensor_mul(ezt[:], et[:], skzb)
                        tiles.append((ezt, ez_mt))
                    for src, dst_mt in tiles:
                        # transpose into [s-slot, t] staging (full 256 slots,
                        # zero-padded), then 2 relayout DMAs -> e table
                        pst = ptp.tile([TBLK, 2 * TBLK], F32, tag="pst", name="pst")
                        stg = pstg.tile([TBLK, 2 * TBLK], F32, tag="stg", name="stg")
                        for h in range(2):
                            s0 = h * TBLK
                            w = min(TBLK, max(0, c.SP - s0))
                            wal = (w // 32) * 32
                            lo = wal
                            while lo < TBLK:   # partition-start rule: 0/32/64/96
                                cnt = {0: 128, 32: 32, 64: 64, 96: 32}[lo]
                                nc.vector.memset(stg[lo:lo + cnt, s0:s0 + TBLK], 0.0)
                                lo += cnt
                            if w > 0:
                                nc.tensor.matmul(pst[:w, s0:s0 + TBLK],
                                                 src[:, s0:s0 + w], sident,
                                                 is_transpose=True)
                                nc.scalar.activation(stg[0:w, s0:s0 + TBLK],
                                                     pst[0:w, s0:s0 + TBLK], CPY)
                        for h in range(2):
                            s0 = h * TBLK
                            c0 = s0 // F
                            dst_ap = dst_mt[mt][b * 32 + c0:b * 32 + c0 + TBLK // F, :] \
                                .rearrange("ch (f t) -> ch f t", t=TBLK)
                            nc.sync.dma_start(dst_ap, stg[0:TBLK, s0:s0 + TBLK])

            # prep must fully land before the DP (collapses wide DMA fan-in
            # to a single sync point; main-MM below overlaps the DP freely)
            if stage >= 3:
                tc.strict_bb_all_engine_barrier()

            # ---- main projection: sum-exp tables ----
            for b in range(BPC):
                for mt in range(c.NMT):
                    idx = b * c.NMT + mt
                    csg = pc.tile([TBLK, NV], F32, tag="csg", name="csg")
                    voff = 0
                    for vc, n in enumerate(c.VCH):
                        psm = pmm.tile([TBLK, 512], F32, tag="psm", name="psm")
                        for k in range(c.KT):
                            nc.tensor.matmul(
                                psm[:, :n], hs_s(b, k, mt),
                                sW[:, k * c.V + voff:k * c.V + voff + n],
                                start=(k == 0), stop=(k == c.KT - 1))
                        nc.scalar.activation(psm[:, :n], psm[:, :n], EXP,
                                             bias=slnc,
                                             accum_out=csg[:, vc:vc + 1])
                        voff += n
                    nc.vector.tensor_reduce(stab[:, idx:idx + 1], csg[:],
                                            axis=AXX, op=ALU.add)

            # ---- DP ----
            x_own = xz[:, 1:1 + F]
            x_halo = xz[:, 0:1]
            x_sh1 = xz[:, 0:F]
            if repeats:
                z_own = xz[:, 2 + F:2 + 2 * F]

            if stage < 4 or (dp_steps is not None and dp_steps < c.T - 1):
                nc.vector.memset(ctab[:], 1.0)
            if stage >= 3:
                e0 = e_mt[0][:].rearrange("p (f t) -> p f t", t=TBLK)[:, :, 0]
                nc.vector.tensor_mul(x_own, e0, sinitm)
                if repeats:
                    ez0 = ez_mt[0][:].rearrange("p (f t) -> p f t", t=TBLK)[:, :, 0]
                    nc.vector.tensor_mul(z_own, ez0, sinitm)

            _tend = c.T if stage >= 4 else 1
            if dp_steps is not None:
                _tend = min(_tend, 1 + dp_steps)
            for t in range(1, _tend):
                mt, tl = divmod(t, TBLK)
                esl = e_mt[mt][:].rearrange("p (f t) -> p f t", t=TBLK)[:, :, tl]
                if not repeats:
                    nc.vector.stream_shuffle(tailt[:], xz[:, F:1 + F], ROT1)
                    nc.vector.tensor_mul(x_halo, tailt[:], rho[:])
                    nc.vector.tensor_add(vt[:], x_own, x_sh1)
                    nc.vector.tensor_add(vt[:, 1:F:2], vt[:, 1:F:2], xz[:, 0:F:2])
                    nc.vector.tensor_mul(x_own, vt[:], esl)
                else:
                    ezsl = ez_mt[mt][:].rearrange("p (f t) -> p f t", t=TBLK)[:, :, tl]
                    nc.vector.stream_shuffle(
                        tailt[:], xz[:, F:2 + 2 * F:F + 1], ROT1)
                    nc.vector.tensor_mul(
                        xz[:, 0:2 + F:1 + F], tailt[:], rho[:].to_broadcast((64, 2)))
                    nc.vector.tensor_add(vt[:], x_own, x_sh1)
                    nc.vector.tensor_add(vt[:, 1:F:2], vt[:, 1:F:2],
                                         xz[:, 1 + F:1 + 2 * F:2])
                    nc.vector.tensor_mul(x_own, vt[:], esl)
                    nc.vector.tensor_mul(z_own, vt[:], ezsl)
                if t % c.RESC == 0:
                    j = t // c.RESC - 1
                    dcol = ctab[:, j:j + 1]
                    nc.vector.tensor_reduce(tot[:], x_own, axis=AXX, op=ALU.add)
                    nc.vector.scalar_tensor_tensor(
                        dcol, tot[:], 0.0, tot[:], op0=ALU.is_le, op1=ALU.add)
                    nc.vector.reciprocal(recip[:], dcol)
                    nc.vector.tensor_scalar_mul(xz[:], xz[:], recip[:])
                    nc.vector.stream_shuffle(dsh[:], dcol, ROT1)
                    nc.vector.scalar_tensor_tensor(
                        rho[:], rho[:], recip[:], dsh[:], op0=ALU.mult, op1=ALU.mult)
                    nc.vector.tensor_scalar_min(rho[:], rho[:], float(c.CLAMP))

            # ---- outputs ----
            nc.sync.dma_start(d_alpha.ap()[:], xz[:])
            nc.sync.dma_start(d_ctab.ap()[:], ctab[:])
            nc.sync.dma_start(d_sums.ap()[:], stab[:])
    nc.finalize()   # bacc compile: wait splitting, reg alloc, nop fusion
    return nc


# ---------------- host side ----------------

def _ext_skip(ys_pad, ys_lens, S):
    Bv = ys_pad.shape[0]
    ext = np.zeros((Bv, S), np.int64)
    ext[:, 1::2] = ys_pad
    ext_m2 = np.concatenate([np.full((Bv, 2), -1), ext[:, :-2]], axis=1)
    skip = (ext != 0) & (ext != ext_m2)
    return ext, skip


def make_core_inputs(cfg, hs_pad, hlens, ys_pad, ys_lens, W, b_bias, repeats):
    c = cfg
    ext, skip = _ext_skip(ys_pad, ys_lens, c.S)
    W16 = W.astype(ml_dtypes.bfloat16)
    Wt = np.ascontiguousarray(W16.reshape(c.KT, TBLK, c.V))
    in_maps = []
    meta = []
    for core in range(NCORES):
        bs = [core * BPC + i for i in range(BPC)]
        hsT = np.zeros((BPC, c.KT, TBLK, c.TP), ml_dtypes.bfloat16)
        Wg = np.zeros((BPC, c.KT, TBLK, c.SP), ml_dtypes.bfloat16)
        tabs = np.zeros((TBLK, c.TW), np.float32)
        tabs[:, c.o_ident:c.o_ident + TBLK] = np.eye(TBLK, dtype=np.float32)
        tabs[:, c.o_lnc] = c.LNC
        for i, b in enumerate(bs):
            ht = hs_pad[b].astype(ml_dtypes.bfloat16)  # [T, D]
            htT = np.zeros((c.D, c.TP), ml_dtypes.bfloat16)
            htT[:, :c.T] = ht.T
            hsT[i] = htT.reshape(c.KT, TBLK, c.TP)
            wg = np.zeros((c.D, c.SP), np.float32)
            wg[:, :c.S] = W[:, ext[b]]
            Wg[i] = wg.astype(ml_dtypes.bfloat16).reshape(c.KT, TBLK, c.SP)
            send = 2 * int(ys_lens[b])
            p = np.zeros(c.SP, np.float32)
            p[0:send + 1:2] = 1.0
            tabs[:, c.o_patt + i * c.SP:c.o_patt + (i + 1) * c.SP] = p[None, :]
            q = np.zeros(c.SP, np.float32)
            q[:send + 1] = 1.0
            tabs[:, c.o_pk + i * c.SP:c.o_pk + (i + 1) * c.SP] = q[None, :]
            z = np.zeros(c.SP, np.float32)
            z[:c.S] = np.concatenate([skip[b][2:].astype(np.float32), [0.0, 0.0]])
            tabs[:, c.o_skz + i * c.SP:c.o_skz + (i + 1) * c.SP] = z[None, :]
            tgrid = np.arange(c.TP)
            tabs[:, c.o_mh + i * c.NMT:c.o_mh + (i + 1) * c.NMT] = (
                tgrid.reshape(c.NMT, TBLK).T < int(hlens[b])).astype(np.float32)
            tabs[i * 32 + 0, c.o_initm + 0] = 1.0
            tabs[i * 32 + 0, c.o_initm + 1] = 1.0
            meta.append(dict(core=core, slot=i, b=b, hlens=int(hlens[b]),
                             send=send))
        tabs[:, c.o_imh:c.o_imh + BPC * c.NMT] = \
            1.0 - tabs[:, c.o_mh:c.o_mh + BPC * c.NMT]
        in_maps.append(dict(hsT=hsT, Wt=Wt, Wg=Wg, tabs=tabs))
    return in_maps, meta


def postprocess(cfg, results, meta, repeats):
    c = cfg
    F = c.F
    total = 0.0
    for info in meta:
        r = results[info["core"]]
        i = info["slot"]
        hl, send = info["hlens"], info["send"]
        alpha = np.asarray(r["alpha_out"], np.float64)
        ctab = np.asarray(r["ctab_out"], np.float64)
        sums = np.asarray(r["sums_out"], np.float64)
        logsig = np.log(ctab[i * 32:(i + 1) * 32, :]).sum(axis=1)  # [32]
        c1, f1 = send // F, send % F
        c0, f0 = (send - 1) // F, (send - 1) % F
        with np.errstate(divide="ignore"):
            la1 = np.log(alpha[i * 32 + c1, 1 + f1]) + logsig[c1]
            la0 = np.log(alpha[i * 32 + c0, 1 + f0]) + logsig[c0]
        la = np.logaddexp(la1, la0)
        st = sums[:, i * c.NMT:(i + 1) * c.NMT].T.reshape(-1)[:hl]
        logZ = np.log(st) - c.LNC
        lb = -(la - logZ.sum() - hl * c.LNC)
        if not (lb < 1e29):
            lb = 0.0
        total += lb
    return np.float32(total / (NCORES * BPC))


_CACHE = {}


def _run(inputs, cfg=FULL, trace=False):
    hs_pad = np.asarray(inputs["hs_pad"], np.float32)
    hlens = np.asarray(inputs["hlens"])
    ys_pad = np.asarray(inputs["ys_pad"])
    ys_lens = np.asarray(inputs["ys_lens"])
    W = np.asarray(inputs["W"], np.float32)
    b_bias = np.asarray(inputs["b"], np.float32)
    repeats = False
    for _b in range(ys_pad.shape[0]):
        _n = int(ys_lens[_b])
        if _n > 1 and bool(np.any(ys_pad[_b, 1:_n] == ys_pad[_b, :_n - 1])):
            repeats = True
            break
    key = (id(cfg), repeats)
    if key not in _CACHE:
        _CACHE[key] = build_program(cfg, repeats)
    nc = _CACHE[key]
    in_maps, meta = make_core_inputs(cfg, hs_pad, hlens, ys_pad, ys_lens, W,
                                     b_bias, repeats)
    res = run_bass_kernel_spmd(nc, in_maps, list(range(NCORES)), trace=trace)
    loss = postprocess(cfg, res.results, meta, repeats)
    return loss, res


def kernel(**inputs) -> np.ndarray:
    loss, _ = _run(inputs)
    return loss



# revision 5
# speedup vs baseline: 1.8444x; 1.8444x over previous
"""Trainium2 Bass kernel for nn_CTC: Linear projection + log_softmax + CTC loss.

Strategy (8 NeuronCores, data-parallel over batch B=16, 2 rows/core):
- Main projection (hs @ W) in bf16 on TensorE with fused ScalarE
  exp-accumulate producing per-frame sum-exp tables (log_softmax
  normalizers); logs and masked sums happen on the host in fp64.
- CTC DP split into a FORWARD chain (alpha, t=0..TM) and a BACKWARD chain
  (e-premultiplied beta, t=T-1..TM+1, stored state-reversed j=206-s so
  both recursions shift the same direction). The two chains for the two
  batch rows pack into the four 32-partition quadrants of the same
  VectorE instructions, halving the serial step count.
- Halo-buffered chunk layout: state s -> partition c=s//16, own lane
  16+s%16; lanes 0..15 replicate the previous chunk's own lanes and
  evolve locally, so per-step work is 2 adds + 1-2 muls with NO
  cross-partition shuffle. The replica window shrinks 2 lanes/step; a
  stream_shuffle + rho-scale refresh every KREF=8 steps restores it.
- Numerical range via per-chunk scales: every RESC=16 steps each chunk
  divides by its own sum (d=1 for dead chunks); rho = sigma_{c-1}/sigma_c
  (clamped, zero-masked at chunk 0 and dead partitions) scales refreshed
  halos. Host reconstructs log-scales from the stored d table.
- For t >= hlens[b] emissions switch to a synthetic blank-pass pattern
  (blank prob 1, labels 0) which exactly preserves the answer for both
  chains. Emissions for states beyond 2*ys_lens[b] are zeroed.
- Host combines: p = sum_s alpha_TM[s] * (b~[s] + b~[s+1] + skip*b~[s+2])
  in fp64 log-space, then folds in logZ and the LNC offsets.

All input-dependent values (masks, label gathers, reversed gathers, init
patterns) enter through per-core data tensors built on the host at call
time; the program itself is uniform SPMD. The bias b is all-zeros by the
problem's input spec and is not applied.
"""

import numpy as np
import ml_dtypes
from dataclasses import dataclass

import concourse.bass as bass
import concourse.bacc as bacc
import concourse.tile as tile
from concourse import mybir
from concourse.bass_utils import run_bass_kernel_spmd

F32 = mybir.dt.float32
BF16 = mybir.dt.bfloat16
ALU = mybir.AluOpType
AXX = mybir.AxisListType.X
EXP = mybir.ActivationFunctionType.Exp
CPY = mybir.ActivationFunctionType.Copy

NCORES = 8
BPC = 2          # batch rows per core
TBLK = 128


@dataclass
class Cfg:
    T: int = 1000
    TP: int = 1024
    D: int = 512
    V: int = 5000
    L: int = 100
    RESC: int = 16
    KREF: int = 8
    LNC: float = -0.9
    CLAMP: float = 1e25
    F: int = 16          # own lanes per chunk
    HL: int = 16         # halo lanes per chunk
    dp_steps: int | None = None   # debug: truncate DP

    @property
    def TM(self):        # fwd steps 1..TM ; bwd covers t=T-1..TM+1
        return (self.T - 2) // 2

    @property
    def NMT(self):
        return self.TP // TBLK

    @property
    def KT(self):
        return self.D // TBLK

    @property
    def S(self):
        return 2 * self.L + 1

    @property
    def SP(self):        # padded states (13 chunks of 16)
        return ((self.S + self.F - 1) // self.F) * self.F

    @property
    def NCH(self):
        return self.SP // self.F

    @property
    def LPP(self):       # lanes per partition per kind
        return self.F + self.HL

    @property
    def NTB(self):       # e-table blocks of TBLK cols covering 0..TM
        return (self.TM + TBLK) // TBLK

    @property
    def VCH(self):
        out = []
        v = self.V
        while v > 0:
            out.append(min(512, v))
            v -= out[-1]
        return out

    @property
    def NEV(self):       # rescale events at i = 17, 33, ... <= TM
        return (self.TM - 1) // self.RESC

    # packed table offsets (fp32 cols in the tabs tensor)
    @property
    def o_pk(self):
        return 0

    @property
    def o_patt(self):
        return 4 * self.SP

    @property
    def o_skz(self):
        return 8 * self.SP

    @property
    def o_mh(self):      # bwd jobs only: (b, mt) for mt in NMT//2-1 .. NMT-1
        return 12 * self.SP

    @property
    def n_bwd_mt(self):
        return self.NMT // 2 + 1

    @property
    def o_imh(self):
        return self.o_mh + BPC * self.n_bwd_mt

    @property
    def o_ident(self):
        return self.o_imh + BPC * self.n_bwd_mt

    @property
    def o_anti(self):
        return self.o_ident + TBLK

    @property
    def o_initm(self):
        return self.o_anti + TBLK

    @property
    def o_rho0(self):
        return self.o_initm + self.LPP

    @property
    def o_lnc(self):
        return self.o_rho0 + 1

    @property
    def TW(self):
        return self.o_lnc + 1


FULL = Cfg()
ROT1 = [(i - 1) % 32 for i in range(32)]


def bwd_pieces(cfg, mt):
    """For bwd source t-block mt (anti-transposed cols u=0..127, table col
    i = T-128-128*mt+u), contiguous pieces (m, dst0, dst1, src0, src1)."""
    base = cfg.T - 128 - 128 * mt
    out = []
    u = 0
    while u < TBLK:
        i = base + u
        if i < 0:
            u = -base
            continue
        if i > cfg.TM:
            break
        m = i // TBLK
        iend = min((m + 1) * TBLK - 1, cfg.TM, base + TBLK - 1)
        n = iend - i + 1
        out.append((m, i - m * TBLK, i - m * TBLK + n, u, u + n))
        u += n
    return out


def build_program(cfg: Cfg, repeats: bool) -> bass.Bass:
    c = cfg
    SP, LPP, NCH = c.SP, c.LPP, c.NCH
    NV = len(c.VCH)
    nc = bacc.Bacc("TRN2", debug=False)

    d_hsT = nc.dram_tensor("hsT", [BPC, c.KT, TBLK, c.TP], BF16, kind="ExternalInput")
    d_W = nc.dram_tensor("Wt", [c.KT, TBLK, c.V], BF16, kind="ExternalInput")
    d_Wg = nc.dram_tensor("Wg", [4, c.KT, TBLK, SP], BF16, kind="ExternalInput")
    d_tabs = nc.dram_tensor("tabs", [TBLK, c.TW], F32, kind="ExternalInput")
    d_sums = nc.dram_tensor("sums_out", [TBLK, BPC * c.NMT], F32, kind="ExternalOutput")
    d_alpha = nc.dram_tensor("alpha_out", [TBLK, 64], F32, kind="ExternalOutput")
    d_ctab = nc.dram_tensor("ctab_out", [TBLK, c.NEV], F32, kind="ExternalOutput")

    with tile.TileContext(nc) as tc:
        with (
            tc.tile_pool(name="persist", bufs=1) as pp,
            tc.tile_pool(name="etile", bufs=3) as pe,
            tc.tile_pool(name="stgp", bufs=3) as pstg,
            tc.tile_pool(name="csum", bufs=2) as pc,
            tc.tile_pool(name="mmps", bufs=2, space="PSUM") as pmm,
            tc.tile_pool(name="gps", bufs=2, space="PSUM") as pgp,
            tc.tile_pool(name="tps", bufs=2, space="PSUM") as ptp,
        ):
            # ---- persistent SBUF ----
            sW = pp.tile([TBLK, c.KT * c.V], BF16, tag="sW", name="sW")
            shsT = pp.tile([TBLK, BPC * c.KT * c.TP], BF16, tag="shsT", name="shsT")
            sWg = pp.tile([TBLK, 4 * c.KT * SP], BF16, tag="sWg", name="sWg")
            tabs = pp.tile([TBLK, c.TW], F32, tag="tabs", name="tabs")
            EX = [pp.tile([TBLK, LPP * TBLK], BF16, tag=f"EX{m}", name=f"EX{m}")
                  for m in range(c.NTB)]
            EZ = ([pp.tile([TBLK, LPP * TBLK], BF16, tag=f"EZ{m}", name=f"EZ{m}")
                   for m in range(c.NTB)] if repeats else None)
            stab = pp.tile([TBLK, BPC * c.NMT], F32, tag="stab", name="stab")
            xz = pp.tile([TBLK, 64], F32, tag="xz", name="xz")
            vt = pp.tile([TBLK, 32], F32, tag="vt", name="vt")
            rho = pp.tile([TBLK, 1], F32, tag="rho", name="rho")
            tot = pp.tile([TBLK, 1], F32, tag="tot", name="tot")
            recip = pp.tile([TBLK, 1], F32, tag="recip", name="recip")
            dsh = pp.tile([TBLK, 1], F32, tag="dsh", name="dsh")
            ctab = pp.tile([TBLK, c.NEV], F32, tag="ctab", name="ctab")

            sident = tabs[:, c.o_ident:c.o_ident + TBLK]
            santi = tabs[:, c.o_anti:c.o_anti + TBLK]
            sinitm = tabs[:, c.o_initm:c.o_initm + LPP]
            slnc = tabs[:, c.o_lnc:c.o_lnc + 1]

            # ---- zero e-tables (dead partitions / unwritten cols), load ----
            for m in range(c.NTB):
                nc.vector.memset(EX[m][:], 0.0)
                if repeats:
                    nc.vector.memset(EZ[m][:], 0.0)
            nc.sync.dma_start(tabs[:], d_tabs.ap()[:])
            for k in range(c.KT):
                nc.sync.dma_start(sW[:, k * c.V:(k + 1) * c.V], d_W.ap()[k])
            for b in range(BPC):
                for k in range(c.KT):
                    off = (b * c.KT + k)
                    nc.sync.dma_start(shsT[:, off * c.TP:(off + 1) * c.TP],
                                      d_hsT.ap()[b, k])
            for q in range(4):
                for k in range(c.KT):
                    off = (q * c.KT + k)
                    nc.sync.dma_start(sWg[:, off * SP:(off + 1) * SP],
                                      d_Wg.ap()[q, k])
            nc.vector.memset(xz[:], 0.0)

            def hs_s(b, k, mt):
                off = (b * c.KT + k) * c.TP + mt * TBLK
                return shsT[:, off:off + TBLK]

            # ---- emission prep: jobs (q, mt) ----
            jobs = [(q, mt) for q in (0, 1) for mt in range(c.NMT // 2)]
            jobs += [(q, mt) for q in (2, 3)
                     for mt in range(c.NMT // 2 - 1, c.NMT)]

            for q, mt in jobs:
                b = q % 2
                fwd = q < 2
                psg = pgp.tile([TBLK, SP], F32, tag="psg", name="psg")
                for k in range(c.KT):
                    off = (q * c.KT + k) * SP
                    nc.tensor.matmul(psg[:], hs_s(b, k, mt),
                                     sWg[:, off:off + SP],
                                     start=(k == 0), stop=(k == c.KT - 1))
                et = pe.tile([TBLK, SP], F32, tag="et", name="et")
                nc.scalar.activation(et[:], psg[:], EXP, bias=slnc)
                pkq = tabs[:, c.o_pk + q * SP:c.o_pk + (q + 1) * SP]
                if fwd:
                    nc.vector.tensor_mul(et[:], et[:], pkq)
                else:
                    ji = b * c.n_bwd_mt + (mt - (c.NMT // 2 - 1))
                    pattq = tabs[:, c.o_patt + q * SP:c.o_patt + (q + 1) * SP]
                    nc.vector.scalar_tensor_tensor(
                        et[:], et[:], tabs[:, c.o_mh + ji:c.o_mh + ji + 1],
                        pkq, op0=ALU.mult, op1=ALU.mult)
                    nc.vector.scalar_tensor_tensor(
                        et[:], pattq, tabs[:, c.o_imh + ji:c.o_imh + ji + 1],
                        et[:], op0=ALU.mult, op1=ALU.add)
                tiles = [(et, EX)]
                if repeats:
                    ezt = pe.tile([TBLK, SP], F32, tag="ezt", name="ezt")
                    skzq = tabs[:, c.o_skz + q * SP:c.o_skz + (q + 1) * SP]
                    nc.vector.tensor_mul(ezt[:], et[:], skzq)
                    tiles.append((ezt, EZ))
                pieces = ([(mt, 0, TBLK, 0, TBLK)] if fwd
                          else bwd_pieces(c, mt))
                for src, dst_mt in tiles:
                    stgs = []
                    for h in range(2):
                        s0 = h * TBLK
                        w = min(TBLK, SP - s0)
                        pst = ptp.tile([TBLK, TBLK], F32, tag="pst", name="pst")
                        stg = pstg.tile([TBLK, TBLK], BF16, tag="stg", name="stg")
                        nc.tensor.matmul(pst[:w, :], src[:, s0:s0 + w],
                                         sident if fwd else santi,
                                         is_transpose=True)
                        nc.scalar.activation(stg[0:w, :], pst[0:w, :], CPY)
                        stgs.append(stg)
                    for m, d0, d1, u0, u1 in pieces:
                        dst = dst_mt[m].rearrange("p (l t) -> p l t", t=TBLK)
                        qb = q * 32
                        nco = c.F  # own lanes start
                        # (src stays a plain 2D slice; dma_start linearizes
                        # both sides in the same element order)
                        # half0 own: chunks 0..7 <- stg0 rows 0..127
                        nc.sync.dma_start(
                            dst[qb:qb + 8, nco:nco + c.F, d0:d1],
                            stgs[0][0:128, u0:u1])
                        # half1 own: chunks 8..12 <- stg1 rows 0..79
                        nc.sync.dma_start(
                            dst[qb + 8:qb + NCH, nco:nco + c.F, d0:d1],
                            stgs[1][0:(NCH - 8) * c.F, u0:u1])
                        # half0 halo: chunks 1..8 <- stg0 rows 0..127
                        nc.sync.dma_start(
                            dst[qb + 1:qb + 9, 0:c.HL, d0:d1],
                            stgs[0][0:128, u0:u1])
                        # half1 halo: chunks 9..12 <- stg1 rows 0..63
                        nc.sync.dma_start(
                            dst[qb + 9:qb + NCH, 0:c.HL, d0:d1],
                            stgs[1][0:(NCH - 9) * c.F, u0:u1])

            # prep must fully land before the DP (collapses wide DMA fan-in
            # to a single sync point; main-MM below overlaps the DP freely)
            tc.strict_bb_all_engine_barrier()

            # ---- main projection: sum-exp tables ----
            for b in range(BPC):
                for mt in range(c.NMT):
                    idx = b * c.NMT + mt
                    csg = pc.tile([TBLK, NV], F32, tag="csg", name="csg")
                    voff = 0
                    for vc, n in enumerate(c.VCH):
                        psm = pmm.tile([TBLK, 512], F32, tag="psm", name="psm")
                        for k in range(c.KT):
                            nc.tensor.matmul(
                                psm[:, :n], hs_s(b, k, mt),
                                sW[:, k * c.V + voff:k * c.V + voff + n],
                                start=(k == 0), stop=(k == c.KT - 1))
                        nc.scalar.activation(psm[:, :n], psm[:, :n], EXP,
                                             bias=slnc,
                                             accum_out=csg[:, vc:vc + 1])
                        voff += n
                    nc.vector.tensor_reduce(stab[:, idx:idx + 1], csg[:],
                                            axis=AXX, op=ALU.add)

            # ---- DP ----
            # xz cols: [x: halo 0..HL, own HL..32 | z: halo 32..48, own 48..64]
            F_, HL_ = c.F, c.HL
            xv = xz[:, 0:32]
            zv = xz[:, 32:64]
            xz4 = xz[:].rearrange("p (a l) -> p a l", l=32)
            tend = c.TM if c.dp_steps is None else min(c.TM, c.dp_steps)
            if c.dp_steps is not None:
                nc.vector.memset(ctab[:], 1.0)
            nc.vector.tensor_copy(rho[:], tabs[:, c.o_rho0:c.o_rho0 + 1])

            def ecol(tbl, m, lo, tl):
                return tbl[m][:].rearrange(
                    "p (l t) -> p l t", t=TBLK)[:, lo:LPP, tl]

            # init from table col 0
            nc.vector.tensor_mul(xv, ecol(EX, 0, 0, 0), sinitm)
            if repeats:
                nc.vector.tensor_mul(zv, ecol(EZ, 0, 0, 0), sinitm)

            nzw = 2 if repeats else 1
            for i in range(1, tend + 1):
                if (i - 1) % c.KREF == 0:
                    if i > 1 and (i - 1) % c.RESC == 0:
                        j = (i - 1) // c.RESC - 1
                        dcol = ctab[:, j:j + 1]
                        nc.vector.tensor_reduce(tot[:], xz[:, HL_:32],
                                                axis=AXX, op=ALU.add)
                        nc.vector.scalar_tensor_tensor(
                            dcol, tot[:], 0.0, tot[:],
                            op0=ALU.is_le, op1=ALU.add)
                        nc.vector.reciprocal(recip[:], dcol)
                        nc.vector.tensor_scalar_mul(
                            xz4[:, 0:nzw, HL_:32], xz4[:, 0:nzw, HL_:32],
                            recip[:])
                        nc.vector.stream_shuffle(dsh[:], dcol, ROT1)
                        nc.vector.scalar_tensor_tensor(
                            rho[:], rho[:], recip[:], dsh[:],
                            op0=ALU.mult, op1=ALU.mult)
                        nc.vector.tensor_scalar_min(rho[:], rho[:],
                                                    float(c.CLAMP))
                    # refresh halos from previous chunk's own lanes
                    nc.vector.stream_shuffle(
                        xz4[:, 0:nzw, 0:HL_], xz4[:, 0:nzw, HL_:32], ROT1)
                    nc.vector.tensor_scalar_mul(
                        xz4[:, 0:nzw, 0:HL_], xz4[:, 0:nzw, 0:HL_], rho[:])
                r = (i - 1) % c.KREF + 1
                lo = 2 * r
                mt, tl = divmod(i, TBLK)
                nc.vector.tensor_add(vt[:, lo:32], xz[:, lo:32],
                                     xz[:, lo - 1:31])
                if repeats:
                    nc.vector.tensor_add(vt[:, lo + 1:32:2],
                                         vt[:, lo + 1:32:2],
                                         xz[:, 32 + lo - 1:63:2])
                else:
                    nc.vector.tensor_add(vt[:, lo + 1:32:2],
                                         vt[:, lo + 1:32:2],
                                         xz[:, lo - 1:31:2])
                nc.vector.tensor_mul(xz[:, lo:32], vt[:, lo:32],
                                     ecol(EX, mt, lo, tl))
                if repeats:
                    nc.vector.tensor_mul(xz[:, 32 + lo:64], vt[:, lo:32],
                                         ecol(EZ, mt, lo, tl))

            # ---- outputs ----
            nc.sync.dma_start(d_alpha.ap()[:], xz[:])
            nc.sync.dma_start(d_ctab.ap()[:], ctab[:])
            nc.sync.dma_start(d_sums.ap()[:], stab[:])
    nc.finalize()   # bacc compile: wait splitting, reg alloc, nop fusion
    return nc


# ---------------- host side ----------------

def _ext_skip(ys_pad, ys_lens, S):
    Bv = ys_pad.shape[0]
    ext = np.zeros((Bv, S), np.int64)
    ext[:, 1::2] = ys_pad
    ext_m2 = np.concatenate([np.full((Bv, 2), -1), ext[:, :-2]], axis=1)
    skip = (ext != 0) & (ext != ext_m2)
    return ext, skip


def make_core_inputs(cfg, hs_pad, hlens, ys_pad, ys_lens, W, b_bias, repeats):
    c = cfg
    S, SP = c.S, c.SP
    ext, skip = _ext_skip(ys_pad, ys_lens, S)
    W16 = W.astype(ml_dtypes.bfloat16)
    Wt = np.ascontiguousarray(W16.reshape(c.KT, TBLK, c.V))
    jrev = 206 - np.arange(SP)   # j index -> original state s (may be <0)
    in_maps = []
    meta = []
    for core in range(NCORES):
        bs = [core * BPC + i for i in range(BPC)]
        hsT = np.zeros((BPC, c.KT, TBLK, c.TP), ml_dtypes.bfloat16)
        Wg = np.zeros((4, c.KT, TBLK, SP), ml_dtypes.bfloat16)
        tabs = np.zeros((TBLK, c.TW), np.float32)
        tabs[:, c.o_ident:c.o_ident + TBLK] = np.eye(TBLK, dtype=np.float32)
        tabs[:, c.o_anti:c.o_anti + TBLK] = np.eye(TBLK,
                                                   dtype=np.float32)[::-1]
        tabs[:, c.o_lnc] = c.LNC
        for i, b in enumerate(bs):
            hl = int(hlens[b])
            send = 2 * int(ys_lens[b])
            ht = hs_pad[b].astype(ml_dtypes.bfloat16)  # [T, D]
            htT = np.zeros((c.D, c.TP), ml_dtypes.bfloat16)
            htT[:, :c.T] = ht.T
            hsT[i] = htT.reshape(c.KT, TBLK, c.TP)
            # gathered weight columns: fwd (q=i) in s-coords, bwd (q=2+i)
            # in reversed j-coords
            wgf = np.zeros((c.D, SP), np.float32)
            wgf[:, :S] = W[:, ext[b]]
            Wg[i] = wgf.astype(ml_dtypes.bfloat16).reshape(c.KT, TBLK, SP)
            wgb = np.zeros((c.D, SP), np.float32)
            okj = (jrev >= 0) & (jrev < S)
            wgb[:, okj] = W[:, ext[b][jrev[okj]]]
            Wg[2 + i] = wgb.astype(ml_dtypes.bfloat16).reshape(c.KT, TBLK, SP)
            # pk / patt / skz per quadrant
            srange = np.arange(SP)
            pkf = ((srange < S) & (srange <= send)).astype(np.float32)
            pattf = pkf * (srange % 2 == 0)
            skzf = np.zeros(SP, np.float32)
            skzf[:S - 2] = skip[b][2:].astype(np.float32)
            pkb = (okj & (jrev <= send)).astype(np.float32)
            pattb = pkb * (jrev % 2 == 0)
            skzb = np.zeros(SP, np.float32)
            skzb[okj] = skip[b][jrev[okj]].astype(np.float32)
            for q, (pk, pt, sk) in ((i, (pkf, pattf, skzf)),
                                    (2 + i, (pkb, pattb, skzb))):
                tabs[:, c.o_pk + q * SP:c.o_pk + (q + 1) * SP] = pk[None, :]
                tabs[:, c.o_patt + q * SP:c.o_patt + (q + 1) * SP] = pt[None, :]
                tabs[:, c.o_skz + q * SP:c.o_skz + (q + 1) * SP] = sk[None, :]
            # bwd time masks (t < hl) per source block
            for n, mt in enumerate(range(c.NMT // 2 - 1, c.NMT)):
                ji = i * c.n_bwd_mt + n
                trow = mt * TBLK + np.arange(TBLK)
                tabs[:, c.o_mh + ji] = (trow < hl).astype(np.float32)
                tabs[:, c.o_imh + ji] = (trow >= hl).astype(np.float32)
            # init masks: fwd states {0,1}; bwd j in {206-send, 207-send}
            for s in (0, 1):
                tabs[i * 32 + s // c.F, c.o_initm + c.HL + s % c.F] = 1.0
            for j in (206 - send, 207 - send):
                tabs[(2 + i) * 32 + j // c.F,
                     c.o_initm + c.HL + j % c.F] = 1.0
            meta.append(dict(core=core, slot=i, b=b, hlens=hl, send=send))
        # rho mask: 1 at live chunks c>=1 of every quadrant
        for q in range(4):
            for ch in range(1, c.NCH):
                tabs[q * 32 + ch, c.o_rho0] = 1.0
        in_maps.append(dict(hsT=hsT, Wt=Wt, Wg=Wg, tabs=tabs))
    return in_maps, meta


def postprocess(cfg, results, meta, skip_all):
    c = cfg
    S = c.S
    total = 0.0
    for info in meta:
        r = results[info["core"]]
        i = info["slot"]
        hl, send, b = info["hlens"], info["send"], info["b"]
        alpha = np.asarray(r["alpha_out"], np.float64)
        ctabv = np.asarray(r["ctab_out"], np.float64)
        sums = np.asarray(r["sums_out"], np.float64)
        skip = skip_all[b]
        with np.errstate(divide="ignore"):
            lsf = np.log(ctabv[i * 32:i * 32 + c.NCH, :]).sum(axis=1)
            lsb = np.log(ctabv[(2 + i) * 32:(2 + i) * 32 + c.NCH, :]).sum(axis=1)
            alog = np.full(S, -np.inf)
            blog = np.full(S + 2, -np.inf)
            for s in range(S):
                cch, f = s // c.F, s % c.F
                alog[s] = np.log(alpha[i * 32 + cch, c.HL + f]) + lsf[cch]
                j = 206 - s
                jc, jf = j // c.F, j % c.F
                blog[s] = np.log(alpha[(2 + i) * 32 + jc, c.HL + jf]) + lsb[jc]
        best = -np.inf
        terms = []
        for s in range(S):
            if not np.isfinite(alog[s]):
                continue
            cands = [blog[s], blog[s + 1]]
            if s + 2 < S and skip[s + 2]:
                cands.append(blog[s + 2])
            m = max(cands)
            if not np.isfinite(m):
                continue
            t = alog[s] + m + np.log(sum(np.exp(x - m) for x in cands))
            terms.append(t)
            best = max(best, t)
        if terms and np.isfinite(best):
            logp = best + np.log(sum(np.exp(t - best) for t in terms))
        else:
            logp = -np.inf
        st = sums[:, i * c.NMT:(i + 1) * c.NMT].T.reshape(-1)[:hl]
        logZ = np.log(st) - c.LNC
        lb = -(logp - hl * c.LNC - logZ.sum())
        if not (lb < 1e29):
            lb = 0.0
        total += lb
    return np.float32(total / (NCORES * BPC))


_CACHE = {}


def _run(inputs, cfg=FULL, trace=False):
    hs_pad = np.asarray(inputs["hs_pad"], np.float32)
    hlens = np.asarray(inputs["hlens"])
    ys_pad = np.asarray(inputs["ys_pad"])
    ys_lens = np.asarray(inputs["ys_lens"])
    W = np.asarray(inputs["W"], np.float32)
    b_bias = np.asarray(inputs["b"], np.float32)
    repeats = False
    for _b in range(ys_pad.shape[0]):
        _n = int(ys_lens[_b])
        if _n > 1 and bool(np.any(ys_pad[_b, 1:_n] == ys_pad[_b, :_n - 1])):
            repeats = True
            break
    key = (id(cfg), repeats)
    if key not in _CACHE:
        _CACHE[key] = build_program(cfg, repeats)
    nc = _CACHE[key]
    in_maps, meta = make_core_inputs(cfg, hs_pad, hlens, ys_pad, ys_lens, W,
                                     b_bias, repeats)
    _, skip_all = _ext_skip(ys_pad, ys_lens, cfg.S)
    res = run_bass_kernel_spmd(nc, in_maps, list(range(NCORES)), trace=trace)
    loss = postprocess(cfg, res.results, meta, skip_all)
    return loss, res


def kernel(**inputs) -> np.ndarray:
    loss, _ = _run(inputs)
    return loss
